# revision 14
# baseline (speedup 1.0000x reference)
"""Trainium2 Bass kernel for nn_CriterionLP (hardest-pos/hardest-neg LP loss).

Math (reference):
    sim  = feats @ feats_s.reshape(B*TOPK, C).T          # [B, B*TOPK]
    blk  = exp(sim/T).reshape(B, P_IDS, K_INST*TOPK)
    pos  = min over own identity block                    # exp is monotone =>
    nmax = max over each identity block                   #   reduce raw sim, exp later
    loss = mean(-log(pos / (pos + sum_{j!=pid} nmax_j + eps) + eps))

Device strategy (8 NeuronCores, SPMD — one program, per-core data):
  * Shard the support dim: core c owns support columns [4096c, 4096(c+1))
    (= identity blocks [32c, 32c+32)); each core sees all B anchors.
  * Anchors are rotated by 512c per core so each core's "own block" diagonal
    band sits at local anchor tiles 0..3 / local blocks [8a, 8a+8) — the
    program is identical across cores.
  * Per core: [C=128 x 4096] @ [C=128 x 4096] fp32r matmuls into PSUM,
    DVE segmented tensor_reduce (max per 128-col identity block; min on the
    diagonal band only), DMA [4096, 32] block-max + [512, 8] band-min out.
  * Host: gather, exp at the [B, 256] level, assemble the scalar loss.
"""

import numpy as np

B = 4096
C = 128
TOPK = 8
K_INST = 16
P_IDS = B // K_INST            # 256 identity blocks
BLK = K_INST * TOPK            # 128 support cols per identity block
TEMP = 0.05
EPS = 1e-6

N_CORES = 8
S_LOC = B * TOPK // N_CORES    # 4096 support cols per core
NBLK_LOC = S_LOC // BLK        # 32 identity blocks per core
A_ROT = B // N_CORES           # 512: per-core anchor rotation
ATILE = 128                    # anchors per tile (partition dim)
N_ATILES = B // ATILE          # 32
BPT = ATILE // K_INST          # 8 own-band blocks per anchor tile

_CACHE = {}


def _build_program():
    import concourse.tile as tile
    from concourse import bacc, mybir
    from concourse.bass import ds, ts

    nc = bacc.Bacc(
        "TRN2", target_bir_lowering=False, debug=False, num_devices=N_CORES
    )
    f32 = mybir.dt.float32
    f16 = mybir.dt.float16
    X = mybir.AxisListType.X

    ft = nc.dram_tensor("ft", [C, B], f16, kind="ExternalInput").ap()
    st = nc.dram_tensor("st", [C, S_LOC], f16, kind="ExternalInput").ap()
    bmax = nc.dram_tensor("bmax", [B, NBLK_LOC], f32, kind="ExternalOutput").ap()
    bmin = nc.dram_tensor("bmin", [A_ROT, BPT], f32, kind="ExternalOutput").ap()

    # Greedy DVE/ACT load balance (measured ns per half-tile).
    DVE_DIRECT = 2280.0   # tensor_reduce [128, 16x128] from PSUM
    DVE_MIN = 1190.0      # extra band min reduce from PSUM
    DVE_TREE = 1800.0     # 2 fp16 2x folds + final reduce (measured)
    ACT_CAST = 2160.0     # PSUM f32 -> SBUF fp16 copy (measured)

    with tile.TileContext(nc) as tc:
        with (
            tc.tile_pool(name="inp", bufs=1) as inp,
            tc.tile_pool(name="res", bufs=4) as resp,
            tc.tile_pool(name="minres", bufs=2) as minp,
            tc.tile_pool(name="cast", bufs=5) as castp,
            tc.tile_pool(name="tree", bufs=4) as treep,
            tc.tile_pool(name="psum", bufs=2, space="PSUM") as pp,
        ):
            ft_r = inp.tile([C, B], f16)
            st_r = inp.tile([C, S_LOC], f16)
            # First anchor tile needs ft[:, 0:128] and st[:, 0:2048]; emit
            # those pieces first so the pipeline starts as soon as possible.
            nc.sync.dma_start(ft_r[:, ts(0, 512)], ft[:, ts(0, 512)])
            for q in range(4):
                nc.sync.dma_start(
                    st_r[:, ts(q, S_LOC // 4)], st[:, ts(q, S_LOC // 4)]
                )
            for q in range(1, 8):
                nc.sync.dma_start(ft_r[:, ts(q, 512)], ft[:, ts(q, 512)])

            # PE HAM warm-up: ~7us of back-to-back dummy matmuls during the
            # input-DMA window flips the clock gate to 8/8 (2.4 GHz) before
            # the real work starts; steady-state gaps are short enough to
            # stay warm after that. Reads an uninitialized scratch tile (no
            # deps -> scheduled first), writes the first PSUM slot.
            warm = inp.tile([C, 512], f16)
            nc.scalar.memzero(warm[:])
            ps_w = pp.tile([ATILE, 4 * 512], f32, tag="ps")
            for i in range(16):
                nc.tensor.matmul(
                    ps_w[:, ts(i % 4, 512)],
                    warm[:, 0:ATILE],
                    warm[:],
                    start=True,
                    stop=True,
                )

            dve_load, act_load = 0.0, 0.0
            for a in range(N_ATILES):
                res = resp.tile([ATILE, NBLK_LOC], f32)
                for h in range(2):  # two PSUM halves of 2048 support cols
                    ps = pp.tile([ATILE, 4 * 512], f32, tag="ps")
                    for j in range(4):
                        nc.tensor.matmul(
                            ps[:, ts(j, 512)],
                            ft_r[:, ts(a, ATILE)],
                            st_r[:, ds(2048 * h + 512 * j, 512)],
                            start=True,
                            stop=True,
                        )
                    diag = a < 4 and h == a // 2
                    direct = diag or (
                        dve_load + DVE_DIRECT <= act_load + ACT_CAST
                    )
                    if direct:
                        dve_load += DVE_DIRECT + (DVE_MIN if diag else 0.0)
                        nc.vector.tensor_reduce(
                            res[:, ds(16 * h, 16)],
                            ps[:].rearrange("p (b x) -> p b x", x=BLK),
                            axis=X,
                            op=mybir.AluOpType.max,
                        )
                        if diag:
                            # own-block band: local blocks [8a, 8a+8)
                            mres = minp.tile([ATILE, BPT], f32)
                            nc.vector.tensor_reduce(
                                mres[:],
                                ps[:, ds((a % 2) * 1024, 1024)].rearrange(
                                    "p (b x) -> p b x", x=BLK
                                ),
                                axis=X,
                                op=mybir.AluOpType.min,
                            )
                            nc.sync.dma_start(bmin[ts(a, ATILE), :], mres[:])
                    else:
                        act_load += ACT_CAST
                        dve_load += DVE_TREE
                        s = castp.tile([ATILE, 16, BLK], f16)
                        nc.scalar.copy(s[:], ps[:].rearrange("p (b x) -> p b x", x=BLK))
                        # 2 pairwise-max folds (fp16 2x mode) + final reduce
                        cur = s
                        for w in (64, 32):
                            nxt = treep.tile([ATILE, 16, w], f16, tag=f"tree{w}")
                            nc.vector.tensor_tensor(
                                nxt[:],
                                cur[:, :, 0:w],
                                cur[:, :, w : 2 * w],
                                op=mybir.AluOpType.max,
                            )
                            cur = nxt
                        nc.vector.tensor_reduce(
                            res[:, ds(16 * h, 16)],
                            cur[:],
                            axis=X,
                            op=mybir.AluOpType.max,
                        )
                nc.sync.dma_start(bmax[ts(a, ATILE), :], res[:])

    nc.compile()
    return nc


def _get_program():
    if "nc" not in _CACHE:
        _CACHE["nc"] = _build_program()
    return _CACHE["nc"]


def _make_in_maps(feats, feats_s):
    fs = feats_s.reshape(B * TOPK, C)
    in_maps = []
    for c in range(N_CORES):
        ftc = np.ascontiguousarray(np.roll(feats, -A_ROT * c, axis=0).T).astype(
            np.float16
        )
        stc = np.ascontiguousarray(fs[S_LOC * c : S_LOC * (c + 1)].T).astype(
            np.float16
        )
        in_maps.append({"ft": ftc, "st": stc})
    return in_maps


def run_device(feats, feats_s, trace=False, tmpdir=None):
    """Run the SPMD program; return (blk_smax [B, P_IDS], pos_sim [B], raw)."""
    from concourse.bass_utils import run_bass_kernel_spmd

    nc = _get_program()
    in_maps = _make_in_maps(feats, feats_s)
    kw = {}
    if trace:
        kw = dict(trace=True, tmpdir=tmpdir)
    r = run_bass_kernel_spmd(nc, in_maps, list(range(N_CORES)), **kw)

    blk_smax = np.empty((B, P_IDS), np.float64)
    pos_sim = np.empty((B,), np.float64)
    i = np.arange(A_ROT)
    for c in range(N_CORES):
        bm = np.asarray(r.results[c]["bmax"])  # [B, 32], rows rotated by 512c
        blk_smax[:, NBLK_LOC * c : NBLK_LOC * (c + 1)] = np.roll(
            bm, A_ROT * c, axis=0
        )
        mn = np.asarray(r.results[c]["bmin"])  # [512, 8] band mins
        pos_sim[A_ROT * c + i] = mn[i, (i // K_INST) % BPT]
    return blk_smax, pos_sim, r


def _loss_from_reductions(blk_smax, pos_sim, labels):
    e = np.exp(blk_smax / TEMP)             # [B, P_IDS] block max of exp
    own = e[np.arange(B), labels]
    neg = e.sum(axis=1) - own
    pos = np.exp(pos_sim / TEMP)
    loss = -np.log(pos / (pos + neg + EPS) + EPS)
    return np.float32(loss.mean())


def _numpy_fallback(feats, feats_s, labels):
    # Exact mirror of the reference, host-only. Safety net for label
    # patterns other than arange(B)//K_INST (never produced by setup_inputs).
    fs = feats_s.reshape(B * TOPK, C)
    out = np.empty((B,), np.float64)
    sim = feats.astype(np.float64) @ fs.astype(np.float64).T
    e = np.exp(sim / TEMP).reshape(B, P_IDS, BLK)
    pos = e[np.arange(B), labels].min(axis=1)
    bm = e.max(axis=2)
    neg = bm.sum(axis=1) - bm[np.arange(B), labels]
    out = -np.log(pos / (pos + neg + EPS) + EPS)
    return np.float32(out.mean())


def kernel(**inputs):
    feats = np.ascontiguousarray(np.asarray(inputs["feats"], dtype=np.float32))
    feats_s = np.ascontiguousarray(np.asarray(inputs["feats_s"], dtype=np.float32))
    labels = np.asarray(inputs["labels"]).astype(np.int64)

    blk_smax, pos_sim, _ = run_device(feats, feats_s)

    if not np.array_equal(labels, np.arange(B, dtype=np.int64) // K_INST):
        return _numpy_fallback(feats, feats_s, labels)
    return _loss_from_reductions(blk_smax, pos_sim, labels)


# revision 16
# speedup vs baseline: 1.0347x; 1.0347x over previous
"""Trainium2 Bass kernel for nn_CriterionLP (hardest-pos/hardest-neg LP loss).

Math (reference):
    sim  = feats @ feats_s.reshape(B*TOPK, C).T          # [B, B*TOPK]
    blk  = exp(sim/T).reshape(B, P_IDS, K_INST*TOPK)
    pos  = min over own identity block                    # exp is monotone =>
    nmax = max over each identity block                   #   reduce raw sim, exp later
    loss = mean(-log(pos / (pos + sum_{j!=pid} nmax_j + eps) + eps))

Device strategy (8 NeuronCores, SPMD — one program, per-core data):
  * Shard the support dim: core c owns support columns [4096c, 4096(c+1))
    (= identity blocks [32c, 32c+32)); each core sees all B anchors.
  * Anchors are rotated by 512c per core so each core's "own block" diagonal
    band sits at local anchor tiles 0..3 / local blocks [8a, 8a+8) — the
    program is identical across cores.
  * Per core: [C=128 x 4096] @ [C=128 x 4096] fp32r matmuls into PSUM,
    DVE segmented tensor_reduce (max per 128-col identity block; min on the
    diagonal band only), DMA [4096, 32] block-max + [512, 8] band-min out.
  * Host: gather, exp at the [B, 256] level, assemble the scalar loss.
"""

import numpy as np

B = 4096
C = 128
TOPK = 8
K_INST = 16
P_IDS = B // K_INST            # 256 identity blocks
BLK = K_INST * TOPK            # 128 support cols per identity block
TEMP = 0.05
EPS = 1e-6

N_CORES = 8
S_LOC = B * TOPK // N_CORES    # 4096 support cols per core
NBLK_LOC = S_LOC // BLK        # 32 identity blocks per core
A_ROT = B // N_CORES           # 512: per-core anchor rotation
ATILE = 128                    # anchors per tile (partition dim)
N_ATILES = B // ATILE          # 32
BPT = ATILE // K_INST          # 8 own-band blocks per anchor tile

_CACHE = {}

# Greedy DVE/ACT load balance (measured ns per half-tile).
DVE_DIRECT = 2280.0   # tensor_reduce [128, 16x128] from PSUM
DVE_MIN = 1190.0      # extra band min reduce from PSUM
DVE_TREE = 1670.0     # 4 fp16 2x folds to width 8 (host finishes 8->1)
ACT_CAST = 2160.0     # PSUM f32 -> SBUF fp16 copy (measured)


def _schedule():
    """Static DVE/ACT assignment; must match between build and host gather."""
    direct_map = {}
    dve_load, act_load = 0.0, 0.0
    for a in range(N_ATILES):
        for h in range(2):
            diag = a < 4 and h == a // 2
            direct = diag or (dve_load + DVE_DIRECT <= act_load + ACT_CAST)
            if direct:
                dve_load += DVE_DIRECT + (DVE_MIN if diag else 0.0)
            else:
                act_load += ACT_CAST
                dve_load += DVE_TREE
            direct_map[(a, h)] = direct
    return direct_map


def _build_program():
    import concourse.tile as tile
    from concourse import bacc, mybir
    from concourse.bass import ds, ts

    nc = bacc.Bacc(
        "TRN2", target_bir_lowering=False, debug=False, num_devices=N_CORES
    )
    f32 = mybir.dt.float32
    f16 = mybir.dt.float16
    X = mybir.AxisListType.X

    ft = nc.dram_tensor("ft", [C, B], f16, kind="ExternalInput").ap()
    st = nc.dram_tensor("st", [C, S_LOC], f16, kind="ExternalInput").ap()
    bmax = nc.dram_tensor("bmax", [B, NBLK_LOC], f32, kind="ExternalOutput").ap()
    bmin = nc.dram_tensor("bmin", [A_ROT, BPT], f32, kind="ExternalOutput").ap()
    bm8 = nc.dram_tensor("bm8", [B, 2, 16, 8], f16, kind="ExternalOutput").ap()

    direct_map = _schedule()

    with tile.TileContext(nc) as tc:
        with (
            tc.tile_pool(name="inp", bufs=1) as inp,
            tc.tile_pool(name="res", bufs=4) as resp,
            tc.tile_pool(name="minres", bufs=2) as minp,
            tc.tile_pool(name="cast", bufs=5) as castp,
            tc.tile_pool(name="tree", bufs=4) as treep,
            tc.tile_pool(name="psum", bufs=2, space="PSUM") as pp,
        ):
            ft_r = inp.tile([C, B], f16)
            st_r = inp.tile([C, S_LOC], f16)
            # First anchor tile needs ft[:, 0:128] and st[:, 0:2048]; emit
            # those pieces first so the pipeline starts as soon as possible.
            nc.sync.dma_start(ft_r[:, ts(0, 512)], ft[:, ts(0, 512)])
            for q in range(4):
                nc.sync.dma_start(
                    st_r[:, ts(q, S_LOC // 4)], st[:, ts(q, S_LOC // 4)]
                )
            for q in range(1, 8):
                nc.sync.dma_start(ft_r[:, ts(q, 512)], ft[:, ts(q, 512)])

            # PE HAM warm-up: ~7us of back-to-back dummy matmuls during the
            # input-DMA window flips the clock gate to 8/8 (2.4 GHz) before
            # the real work starts; steady-state gaps are short enough to
            # stay warm after that. Reads an uninitialized scratch tile (no
            # deps -> scheduled first), writes the first PSUM slot.
            warm = inp.tile([C, 512], f16)
            nc.scalar.memzero(warm[:])
            ps_w = pp.tile([ATILE, 4 * 512], f32, tag="ps")
            for i in range(10):
                nc.tensor.matmul(
                    ps_w[:, ts(i % 4, 512)],
                    warm[:, 0:ATILE],
                    warm[:],
                    start=True,
                    stop=True,
                )

            for a in range(N_ATILES):
                for h in range(2):  # two PSUM halves of 2048 support cols
                    ps = pp.tile([ATILE, 4 * 512], f32, tag="ps")
                    for j in range(4):
                        nc.tensor.matmul(
                            ps[:, ts(j, 512)],
                            ft_r[:, ts(a, ATILE)],
                            st_r[:, ds(2048 * h + 512 * j, 512)],
                            start=True,
                            stop=True,
                        )
                    diag = a < 4 and h == a // 2
                    if direct_map[(a, h)]:
                        res = resp.tile([ATILE, 16], f32)
                        nc.vector.tensor_reduce(
                            res[:],
                            ps[:].rearrange("p (b x) -> p b x", x=BLK),
                            axis=X,
                            op=mybir.AluOpType.max,
                        )
                        nc.sync.dma_start(
                            bmax[ts(a, ATILE), ds(16 * h, 16)], res[:]
                        )
                        if diag:
                            # own-block band: local blocks [8a, 8a+8)
                            mres = minp.tile([ATILE, BPT], f32)
                            nc.vector.tensor_reduce(
                                mres[:],
                                ps[:, ds((a % 2) * 1024, 1024)].rearrange(
                                    "p (b x) -> p b x", x=BLK
                                ),
                                axis=X,
                                op=mybir.AluOpType.min,
                            )
                            nc.sync.dma_start(bmin[ts(a, ATILE), :], mres[:])
                    else:
                        s = castp.tile([ATILE, 16, BLK], f16)
                        nc.scalar.copy(s[:], ps[:].rearrange("p (b x) -> p b x", x=BLK))
                        # fp16 2x pairwise-max folds down to width 8;
                        # the host finishes the last 8->1 reduction.
                        cur = s
                        for w in (64, 32, 16, 8):
                            nxt = treep.tile([ATILE, 16, w], f16, tag=f"tree{w}")
                            nc.vector.tensor_tensor(
                                nxt[:],
                                cur[:, :, 0:w],
                                cur[:, :, w : 2 * w],
                                op=mybir.AluOpType.max,
                            )
                            cur = nxt
                        nc.sync.dma_start(bm8[ts(a, ATILE), h, :, :], cur[:])

    nc.compile()
    return nc


def _get_program():
    if "nc" not in _CACHE:
        _CACHE["nc"] = _build_program()
    return _CACHE["nc"]


def _make_in_maps(feats, feats_s):
    fs = feats_s.reshape(B * TOPK, C)
    in_maps = []
    for c in range(N_CORES):
        ftc = np.ascontiguousarray(np.roll(feats, -A_ROT * c, axis=0).T).astype(
            np.float16
        )
        stc = np.ascontiguousarray(fs[S_LOC * c : S_LOC * (c + 1)].T).astype(
            np.float16
        )
        in_maps.append({"ft": ftc, "st": stc})
    return in_maps


def run_device(feats, feats_s, trace=False, tmpdir=None):
    """Run the SPMD program; return (blk_smax [B, P_IDS], pos_sim [B], raw)."""
    from concourse.bass_utils import run_bass_kernel_spmd

    nc = _get_program()
    in_maps = _make_in_maps(feats, feats_s)
    kw = {}
    if trace:
        kw = dict(trace=True, tmpdir=tmpdir)
    r = run_bass_kernel_spmd(nc, in_maps, list(range(N_CORES)), **kw)

    direct_map = _schedule()
    blk_smax = np.empty((B, P_IDS), np.float64)
    pos_sim = np.empty((B,), np.float64)
    i = np.arange(A_ROT)
    for c in range(N_CORES):
        bm = np.array(r.results[c]["bmax"])    # [B, 32]; valid on direct halves
        bm8 = np.asarray(r.results[c]["bm8"])  # [B, 2, 16, 8] fp16 tree tops
        bm8 = bm8.astype(np.float32).max(axis=3)  # [B, 2, 16]
        for a in range(N_ATILES):
            for h in range(2):
                if not direct_map[(a, h)]:
                    bm[128 * a : 128 * (a + 1), 16 * h : 16 * (h + 1)] = bm8[
                        128 * a : 128 * (a + 1), h
                    ]
        blk_smax[:, NBLK_LOC * c : NBLK_LOC * (c + 1)] = np.roll(
            bm, A_ROT * c, axis=0
        )
        mn = np.asarray(r.results[c]["bmin"])  # [512, 8] band mins
        pos_sim[A_ROT * c + i] = mn[i, (i // K_INST) % BPT]
    return blk_smax, pos_sim, r


def _loss_from_reductions(blk_smax, pos_sim, labels):
    e = np.exp(blk_smax / TEMP)             # [B, P_IDS] block max of exp
    own = e[np.arange(B), labels]
    neg = e.sum(axis=1) - own
    pos = np.exp(pos_sim / TEMP)
    loss = -np.log(pos / (pos + neg + EPS) + EPS)
    return np.float32(loss.mean())


def _numpy_fallback(feats, feats_s, labels):
    # Exact mirror of the reference, host-only. Safety net for label
    # patterns other than arange(B)//K_INST (never produced by setup_inputs).
    fs = feats_s.reshape(B * TOPK, C)
    out = np.empty((B,), np.float64)
    sim = feats.astype(np.float64) @ fs.astype(np.float64).T
    e = np.exp(sim / TEMP).reshape(B, P_IDS, BLK)
    pos = e[np.arange(B), labels].min(axis=1)
    bm = e.max(axis=2)
    neg = bm.sum(axis=1) - bm[np.arange(B), labels]
    out = -np.log(pos / (pos + neg + EPS) + EPS)
    return np.float32(out.mean())


def kernel(**inputs):
    feats = np.ascontiguousarray(np.asarray(inputs["feats"], dtype=np.float32))
    feats_s = np.ascontiguousarray(np.asarray(inputs["feats_s"], dtype=np.float32))
    labels = np.asarray(inputs["labels"]).astype(np.int64)

    blk_smax, pos_sim, _ = run_device(feats, feats_s)

    if not np.array_equal(labels, np.arange(B, dtype=np.int64) // K_INST):
        return _numpy_fallback(feats, feats_s, labels)
    return _loss_from_reductions(blk_smax, pos_sim, labels)


# revision 17
# speedup vs baseline: 1.0527x; 1.0174x over previous
"""Trainium2 Bass kernel for nn_CriterionLP (hardest-pos/hardest-neg LP loss).

Math (reference):
    sim  = feats @ feats_s.reshape(B*TOPK, C).T          # [B, B*TOPK]
    blk  = exp(sim/T).reshape(B, P_IDS, K_INST*TOPK)
    pos  = min over own identity block                    # exp is monotone =>
    nmax = max over each identity block                   #   reduce raw sim, exp later
    loss = mean(-log(pos / (pos + sum_{j!=pid} nmax_j + eps) + eps))

Device strategy (8 NeuronCores, SPMD — one program, per-core data):
  * Shard the support dim: core c owns support columns [4096c, 4096(c+1))
    (= identity blocks [32c, 32c+32)); each core sees all B anchors.
  * Anchors are rotated by 512c per core so each core's "own block" diagonal
    band sits at local anchor tiles 0..3 / local blocks [8a, 8a+8) — the
    program is identical across cores.
  * Per core: [C=128 x 4096] @ [C=128 x 4096] fp32r matmuls into PSUM,
    DVE segmented tensor_reduce (max per 128-col identity block; min on the
    diagonal band only), DMA [4096, 32] block-max + [512, 8] band-min out.
  * Host: gather, exp at the [B, 256] level, assemble the scalar loss.
"""

import numpy as np

B = 4096
C = 128
TOPK = 8
K_INST = 16
P_IDS = B // K_INST            # 256 identity blocks
BLK = K_INST * TOPK            # 128 support cols per identity block
TEMP = 0.05
EPS = 1e-6

N_CORES = 8
S_LOC = B * TOPK // N_CORES    # 4096 support cols per core
NBLK_LOC = S_LOC // BLK        # 32 identity blocks per core
A_ROT = B // N_CORES           # 512: per-core anchor rotation
ATILE = 128                    # anchors per tile (partition dim)
N_ATILES = B // ATILE          # 32
BPT = ATILE // K_INST          # 8 own-band blocks per anchor tile

_CACHE = {}

# Greedy DVE/ACT load balance (measured ns per half-tile).
DVE_DIRECT = 2280.0   # tensor_reduce [128, 16x128] from PSUM
DVE_MIN = 1190.0      # extra band min reduce from PSUM
DVE_TREE = 1670.0     # 4 fp16 2x folds to width 8 (host finishes 8->1)
ACT_CAST = 2160.0     # PSUM f32 -> SBUF fp16 copy (measured)


def _schedule():
    """Static DVE/ACT assignment; must match between build and host gather."""
    direct_map = {}
    dve_load, act_load = 0.0, 0.0
    for a in range(N_ATILES):
        for h in range(2):
            diag = a < 4 and h == a // 2
            direct = diag or (dve_load + DVE_DIRECT <= act_load + ACT_CAST)
            if direct:
                dve_load += DVE_DIRECT + (DVE_MIN if diag else 0.0)
            else:
                act_load += ACT_CAST
                dve_load += DVE_TREE
            direct_map[(a, h)] = direct
    return direct_map


def _build_program():
    import concourse.tile as tile
    from concourse import bacc, mybir
    from concourse.bass import ds, ts

    nc = bacc.Bacc(
        "TRN2", target_bir_lowering=False, debug=False, num_devices=N_CORES
    )
    f32 = mybir.dt.float32
    f16 = mybir.dt.float16
    X = mybir.AxisListType.X

    ft = nc.dram_tensor("ft", [C, B], f16, kind="ExternalInput").ap()
    st = nc.dram_tensor("st", [C, S_LOC], f16, kind="ExternalInput").ap()
    bmax = nc.dram_tensor("bmax", [B, NBLK_LOC], f32, kind="ExternalOutput").ap()
    bmin = nc.dram_tensor("bmin", [A_ROT, BPT], f32, kind="ExternalOutput").ap()
    bm8 = nc.dram_tensor("bm8", [B, 2, 16, 8], f16, kind="ExternalOutput").ap()

    direct_map = _schedule()

    with tile.TileContext(nc) as tc:
        with (
            tc.tile_pool(name="inp", bufs=1) as inp,
            tc.tile_pool(name="res", bufs=4) as resp,
            tc.tile_pool(name="minres", bufs=2) as minp,
            tc.tile_pool(name="cast", bufs=7) as castp,
            tc.tile_pool(name="tree", bufs=6) as treep,
            tc.tile_pool(name="psum", bufs=2, space="PSUM") as pp,
        ):
            ft_r = inp.tile([C, B], f16)
            st_r = inp.tile([C, S_LOC], f16)
            # First anchor tile needs ft[:, 0:128] and st[:, 0:2048]; emit
            # those pieces first so the pipeline starts as soon as possible.
            nc.sync.dma_start(ft_r[:, ts(0, 512)], ft[:, ts(0, 512)])
            for q in range(4):
                nc.sync.dma_start(
                    st_r[:, ts(q, S_LOC // 4)], st[:, ts(q, S_LOC // 4)]
                )
            for q in range(1, 8):
                nc.sync.dma_start(ft_r[:, ts(q, 512)], ft[:, ts(q, 512)])

            # PE HAM warm-up: ~7us of back-to-back dummy matmuls during the
            # input-DMA window flips the clock gate to 8/8 (2.4 GHz) before
            # the real work starts; steady-state gaps are short enough to
            # stay warm after that. Reads an uninitialized scratch tile (no
            # deps -> scheduled first), writes the first PSUM slot.
            warm = inp.tile([C, 512], f16)
            nc.scalar.memzero(warm[:])
            ps_w = pp.tile([ATILE, 4 * 512], f32, tag="ps")
            for i in range(7):
                nc.tensor.matmul(
                    ps_w[:, ts(i % 4, 512)],
                    warm[:, 0:ATILE],
                    warm[:],
                    start=True,
                    stop=True,
                )

            for a in range(N_ATILES):
                for h in range(2):  # two PSUM halves of 2048 support cols
                    ps = pp.tile([ATILE, 4 * 512], f32, tag="ps")
                    for j in range(4):
                        nc.tensor.matmul(
                            ps[:, ts(j, 512)],
                            ft_r[:, ts(a, ATILE)],
                            st_r[:, ds(2048 * h + 512 * j, 512)],
                            start=True,
                            stop=True,
                        )
                    diag = a < 4 and h == a // 2
                    if direct_map[(a, h)]:
                        res = resp.tile([ATILE, 16], f32)
                        nc.vector.tensor_reduce(
                            res[:],
                            ps[:].rearrange("p (b x) -> p b x", x=BLK),
                            axis=X,
                            op=mybir.AluOpType.max,
                        )
                        nc.sync.dma_start(
                            bmax[ts(a, ATILE), ds(16 * h, 16)], res[:]
                        )
                        if diag:
                            # own-block band: local blocks [8a, 8a+8)
                            mres = minp.tile([ATILE, BPT], f32)
                            nc.vector.tensor_reduce(
                                mres[:],
                                ps[:, ds((a % 2) * 1024, 1024)].rearrange(
                                    "p (b x) -> p b x", x=BLK
                                ),
                                axis=X,
                                op=mybir.AluOpType.min,
                            )
                            nc.sync.dma_start(bmin[ts(a, ATILE), :], mres[:])
                    else:
                        s = castp.tile([ATILE, 16, BLK], f16)
                        nc.scalar.copy(s[:], ps[:].rearrange("p (b x) -> p b x", x=BLK))
                        # fp16 2x pairwise-max folds down to width 8;
                        # the host finishes the last 8->1 reduction.
                        cur = s
                        for w in (64, 32, 16, 8):
                            nxt = treep.tile([ATILE, 16, w], f16, tag=f"tree{w}")
                            nc.vector.tensor_tensor(
                                nxt[:],
                                cur[:, :, 0:w],
                                cur[:, :, w : 2 * w],
                                op=mybir.AluOpType.max,
                            )
                            cur = nxt
                        nc.sync.dma_start(bm8[ts(a, ATILE), h, :, :], cur[:])

    nc.compile()
    return nc


def _get_program():
    if "nc" not in _CACHE:
        _CACHE["nc"] = _build_program()
    return _CACHE["nc"]


def _make_in_maps(feats, feats_s):
    fs = feats_s.reshape(B * TOPK, C)
    in_maps = []
    for c in range(N_CORES):
        ftc = np.ascontiguousarray(np.roll(feats, -A_ROT * c, axis=0).T).astype(
            np.float16
        )
        stc = np.ascontiguousarray(fs[S_LOC * c : S_LOC * (c + 1)].T).astype(
            np.float16
        )
        in_maps.append({"ft": ftc, "st": stc})
    return in_maps


def run_device(feats, feats_s, trace=False, tmpdir=None):
    """Run the SPMD program; return (blk_smax [B, P_IDS], pos_sim [B], raw)."""
    from concourse.bass_utils import run_bass_kernel_spmd

    nc = _get_program()
    in_maps = _make_in_maps(feats, feats_s)
    kw = {}
    if trace:
        kw = dict(trace=True, tmpdir=tmpdir)
    r = run_bass_kernel_spmd(nc, in_maps, list(range(N_CORES)), **kw)

    direct_map = _schedule()
    blk_smax = np.empty((B, P_IDS), np.float64)
    pos_sim = np.empty((B,), np.float64)
    i = np.arange(A_ROT)
    for c in range(N_CORES):
        bm = np.array(r.results[c]["bmax"])    # [B, 32]; valid on direct halves
        bm8 = np.asarray(r.results[c]["bm8"])  # [B, 2, 16, 8] fp16 tree tops
        bm8 = bm8.astype(np.float32).max(axis=3)  # [B, 2, 16]
        for a in range(N_ATILES):
            for h in range(2):
                if not direct_map[(a, h)]:
                    bm[128 * a : 128 * (a + 1), 16 * h : 16 * (h + 1)] = bm8[
                        128 * a : 128 * (a + 1), h
                    ]
        blk_smax[:, NBLK_LOC * c : NBLK_LOC * (c + 1)] = np.roll(
            bm, A_ROT * c, axis=0
        )
        mn = np.asarray(r.results[c]["bmin"])  # [512, 8] band mins
        pos_sim[A_ROT * c + i] = mn[i, (i // K_INST) % BPT]
    return blk_smax, pos_sim, r


def _loss_from_reductions(blk_smax, pos_sim, labels):
    e = np.exp(blk_smax / TEMP)             # [B, P_IDS] block max of exp
    own = e[np.arange(B), labels]
    neg = e.sum(axis=1) - own
    pos = np.exp(pos_sim / TEMP)
    loss = -np.log(pos / (pos + neg + EPS) + EPS)
    return np.float32(loss.mean())


def _numpy_fallback(feats, feats_s, labels):
    # Exact mirror of the reference, host-only. Safety net for label
    # patterns other than arange(B)//K_INST (never produced by setup_inputs).
    fs = feats_s.reshape(B * TOPK, C)
    out = np.empty((B,), np.float64)
    sim = feats.astype(np.float64) @ fs.astype(np.float64).T
    e = np.exp(sim / TEMP).reshape(B, P_IDS, BLK)
    pos = e[np.arange(B), labels].min(axis=1)
    bm = e.max(axis=2)
    neg = bm.sum(axis=1) - bm[np.arange(B), labels]
    out = -np.log(pos / (pos + neg + EPS) + EPS)
    return np.float32(out.mean())


def kernel(**inputs):
    feats = np.ascontiguousarray(np.asarray(inputs["feats"], dtype=np.float32))
    feats_s = np.ascontiguousarray(np.asarray(inputs["feats_s"], dtype=np.float32))
    labels = np.asarray(inputs["labels"]).astype(np.int64)

    blk_smax, pos_sim, _ = run_device(feats, feats_s)

    if not np.array_equal(labels, np.arange(B, dtype=np.int64) // K_INST):
        return _numpy_fallback(feats, feats_s, labels)
    return _loss_from_reductions(blk_smax, pos_sim, labels)
